# revision 1
# baseline (speedup 1.0000x reference)
"""Trainium2 Bass kernel for KroneckerLinear: y = x @ kron(U, V).

Math: with x[t] reshaped to X_t [i1=128, i2=128] (i2 contiguous) and
y[t] reshaped to Y_t [j1=128, j2=128] (j2 contiguous):

    Y_t = U^T @ X_t @ V

Both stages map onto the PE array with the *token* tile as the stationary
operand (lhsT), so every tensor stays in its natural (contiguous) layout
and no transposes are needed anywhere:

    MM1: out = lhsT.T @ rhs with lhsT = X_t  [i1, i2], rhs = U [i1, j1]
         -> P^T [i2, j1]   (P = U^T X_t)
    MM2: lhsT = P^T [i2, j1], rhs = V [i2, j2]
         -> Y_t [j1, j2]

Sharding: data-parallel over the token dim, 256 tokens per core x 8 cores.
"""

import sys

if "/opt/trn_rl_repo" not in sys.path:
    sys.path.insert(0, "/opt/trn_rl_repo")

import numpy as np

import concourse.bacc as bacc
import concourse.bass as bass
import concourse.mybir as mybir
from concourse import tile
from concourse.bass_utils import run_bass_kernel_spmd

F32 = mybir.dt.float32
F32R = mybir.dt.float32r

N_CORES = 8
TOKENS = 2048
D = 16384  # 128 * 128
T_CORE = TOKENS // N_CORES  # 256


def build_nc(n_tokens=T_CORE, mode="fp32r", group=32, quad=4, lgran=0, sgran=0):
    """Build + compile the per-core program.

    mode:
      "fp32"   - exact fp32 matmuls (4 cycles/row on PE)
      "fp32r"  - float32r matmuls with the moving operand padded to 256
                 columns ([U|U], [V|V]) to hit the 1 cycle/row fast path.
                 fp32r operands must be *produced* rounded: x tiles are
                 cast during the (SWDGE) load, P tiles by the DVE copy.
      "fp32rh" - like fp32r but x/U/V are pre-rounded to the fp32r grid
                 on the host (zeroed low 12 mantissa bits), declared as
                 float32r DRAM tensors, and loaded with plain HWDGE DMA.

    lgran/sgran: tokens per load/store dma_start (0 = whole group in one
    strided transfer). gran=1 gives fully contiguous 64 KB transfers,
    which keeps the HBM access stream sequential.
    """
    assert n_tokens % group == 0 and group % quad == 0
    r = mode in ("fp32r", "fp32rh")
    hostround = mode == "fp32rh"
    rwide = 256 if r else 128
    mmdt = F32R if r else F32
    dramdt = F32R if hostround else F32
    lgran = lgran or group
    sgran = sgran or group
    assert group % lgran == 0 and group % sgran == 0

    nc = bacc.Bacc("TRN2", target_bir_lowering=False, debug=False)
    x = nc.dram_tensor("x", [n_tokens, D], dramdt, kind="ExternalInput")
    u = nc.dram_tensor("u", [128, rwide], dramdt, kind="ExternalInput")
    v = nc.dram_tensor("v", [128, rwide], dramdt, kind="ExternalInput")
    y = nc.dram_tensor("y", [n_tokens, D], F32, kind="ExternalOutput")

    with tile.TileContext(nc) as tc:
        with (
            tc.tile_pool(name="const", bufs=1) as cpool,
            tc.tile_pool(name="xin", bufs=2) as xpool,
            tc.tile_pool(name="yout", bufs=2) as ypool,
            tc.tile_pool(name="pmid", bufs=4) as ppool,
            tc.tile_pool(name="ps", bufs=2, space="PSUM") as pspool,
        ):
            u_sb = cpool.tile([128, rwide], mmdt)
            v_sb = cpool.tile([128, rwide], mmdt)
            ld_const = (
                nc.gpsimd.dma_start if (r and not hostround) else nc.sync.dma_start
            )
            ld_const(u_sb[:], u[:])
            ld_const(v_sb[:], v[:])

            for g in range(n_tokens // group):
                xt = xpool.tile([128, group, 128], mmdt)
                ld_x = (
                    nc.gpsimd.dma_start if (r and not hostround) else nc.sync.dma_start
                )
                for c in range(group // lgran):
                    t0 = g * group + c * lgran
                    ld_x(
                        xt[:, c * lgran : (c + 1) * lgran, :],
                        x[t0 : t0 + lgran].rearrange("t (i1 i2) -> i1 t i2", i1=128),
                    )
                yt = ypool.tile([128, group, 128], F32)
                for q in range(group // quad):
                    pa = pspool.tile([128, quad, rwide], F32)
                    for j in range(quad):
                        nc.tensor.matmul(
                            pa[:, j, :],
                            lhsT=xt[:, q * quad + j, :],
                            rhs=u_sb[:],
                            start=True,
                            stop=True,
                        )
                    psb = ppool.tile([128, quad, 128], mmdt)
                    nc.vector.tensor_copy(psb[:], pa[:, :, 0:128])
                    pb = pspool.tile([128, quad, rwide], F32)
                    for j in range(quad):
                        nc.tensor.matmul(
                            pb[:, j, :],
                            lhsT=psb[:, j, :],
                            rhs=v_sb[:],
                            start=True,
                            stop=True,
                        )
                    nc.vector.tensor_copy(
                        yt[:, q * quad : (q + 1) * quad, :], pb[:, :, 0:128]
                    )
                for c in range(group // sgran):
                    t0 = g * group + c * sgran
                    nc.scalar.dma_start(
                        y[t0 : t0 + sgran].rearrange("t (j1 j2) -> j1 t j2", j1=128),
                        yt[:, c * sgran : (c + 1) * sgran, :],
                    )
    nc.compile()
    return nc


_NC_CACHE = {}


def _get_nc(n_tokens, mode, group, quad, lgran, sgran):
    key = (n_tokens, mode, group, quad, lgran, sgran)
    if key not in _NC_CACHE:
        _NC_CACHE[key] = build_nc(n_tokens, mode, group, quad, lgran, sgran)
    return _NC_CACHE[key]


def round_fp32r(a):
    """Round fp32 array to the fp32r grid (11-bit mantissa, round-to-nearest)."""
    u = np.ascontiguousarray(a, dtype=np.float32).view(np.uint32)
    r = ((u + np.uint32(0x800)) & np.uint32(0xFFFFF000)).view(np.float32)
    return np.where(np.isfinite(a), r, a).astype(np.float32)


def _prep_inputs(x, U, V, mode):
    x = np.ascontiguousarray(np.asarray(x), dtype=np.float32)
    U = np.ascontiguousarray(np.asarray(U), dtype=np.float32)
    V = np.ascontiguousarray(np.asarray(V), dtype=np.float32)
    if mode in ("fp32r", "fp32rh"):
        U = np.concatenate([U, U], axis=1)
        V = np.concatenate([V, V], axis=1)
    if mode == "fp32rh":
        x = round_fp32r(x)
        U = round_fp32r(U)
        V = round_fp32r(V)
    return x, U, V


def run(x, U, V, mode="fp32rh", group=32, quad=4, lgran=0, sgran=0,
        trace=False, **spmd_kwargs):
    """Shard over 8 cores, run, gather. Returns (y_full, BassKernelResults)."""
    x, U, V = _prep_inputs(x, U, V, mode)
    t_core = x.shape[0] // N_CORES
    nc = _get_nc(t_core, mode, group, quad, lgran, sgran)
    in_maps = [
        {"x": x[i * t_core : (i + 1) * t_core], "u": U, "v": V}
        for i in range(N_CORES)
    ]
    res = run_bass_kernel_spmd(
        nc, in_maps, list(range(N_CORES)), trace=trace, **spmd_kwargs
    )
    out = np.concatenate([res.results[i]["y"] for i in range(N_CORES)], axis=0)
    return out, res


def kernel(x, U, V):
    out, _ = run(x, U, V)
    return out



# revision 2
# speedup vs baseline: 1.9724x; 1.9724x over previous
"""Trainium2 Bass kernel for KroneckerLinear: y = x @ kron(U, V).

Math: with x[t] reshaped to X_t [i1=128, i2=128] (i2 contiguous) and
y[t] reshaped to Y_t [j1=128, j2=128] (j2 contiguous):

    Y_t = U^T @ X_t @ V

    MM1: out = lhsT.T @ rhs with lhsT = X_t  [i1, i2], rhs = U [i1, j1]
         -> P^T [i2, j1]   (P = U^T X_t)
    MM2: lhsT = P^T [i2, j1], rhs = V [i2, j2]
         -> Y_t [j1, j2]

Everything runs in bf16 (inputs bf16, PSUM accum fp32, intermediate P and
output y rounded to bf16).  End-to-end rel err vs the fp32 reference is
~4.6e-3 (absmax / max|y|), well under the 2e-2 gate.

DMA layout: x is pre-permuted on the host to xs[i1, t, i2] (bf16) so the
device loads are large fully-contiguous descriptors (G*256 B per partition
per group) instead of the 256 B strided chunks a [t, (i1 i2)] layout would
need (descriptors < 512 B run at half DMA throughput).  y is produced on
device as ys[j1, t, j2] (bf16, contiguous stores) and un-permuted on the
host.  This halves HBM traffic vs fp32 while keeping full descriptor
efficiency: ~22 us load + ~22 us store per core at 360 GB/s.

Engine budget per core (256 tokens):
  PE : 2 matmuls/token (bf16, N=128 -> 53 ns) + per-token LDWEIGHTS -> ~41 us
  DVE: P^T PSUM->SBUF bf16 copy  -> ~34 us
  Act: Y   PSUM->SBUF bf16 copy  -> ~27 us
  DMA: 16 MB bf16 traffic        -> ~44 us   <- expected bound

Sharding: data-parallel over the token dim, 256 tokens per core x 8 cores.
"""

import sys

if "/opt/trn_rl_repo" not in sys.path:
    sys.path.insert(0, "/opt/trn_rl_repo")

import ml_dtypes
import numpy as np

import concourse.bacc as bacc
import concourse.mybir as mybir
from concourse import tile
from concourse.bass_utils import run_bass_kernel_spmd

F32 = mybir.dt.float32
BF16 = mybir.dt.bfloat16
NP_BF16 = ml_dtypes.bfloat16

N_CORES = 8
TOKENS = 2048
D = 16384  # 128 * 128
T_CORE = TOKENS // N_CORES  # 256


def build_nc(n_tokens=T_CORE, group=32, quad=4, pipe=True, swap_copy=False):
    """Build + compile the per-core program.

    group: tokens per DMA transfer (load and store granularity).
    quad:  tokens per PSUM tile / per copy instruction.
    pipe:  software-pipeline MM2(q-1) after MM1(q) so the PE never waits
           on the DVE copy of the current quad.
    swap_copy: put the P copy on Act and the Y copy on DVE instead.
    """
    assert n_tokens % group == 0 and group % quad == 0

    nc = bacc.Bacc("TRN2", target_bir_lowering=False, debug=False)
    xs = nc.dram_tensor("xs", [128, n_tokens, 128], BF16, kind="ExternalInput")
    u = nc.dram_tensor("u", [128, 128], BF16, kind="ExternalInput")
    v = nc.dram_tensor("v", [128, 128], BF16, kind="ExternalInput")
    ys = nc.dram_tensor("ys", [128, n_tokens, 128], BF16, kind="ExternalOutput")

    with tile.TileContext(nc) as tc:
        with (
            tc.tile_pool(name="const", bufs=1) as cpool,
            tc.tile_pool(name="xin", bufs=2) as xpool,
            tc.tile_pool(name="yout", bufs=2) as ypool,
            tc.tile_pool(name="pmid", bufs=4) as ppool,
            tc.tile_pool(name="psa", bufs=2, space="PSUM") as pspool_a,
            tc.tile_pool(name="psb", bufs=2, space="PSUM") as pspool_b,
        ):
            u_sb = cpool.tile([128, 128], BF16)
            v_sb = cpool.tile([128, 128], BF16)
            nc.sync.dma_start(u_sb[:], u[:])
            nc.sync.dma_start(v_sb[:], v[:])

            copy_p = nc.scalar.copy if swap_copy else nc.vector.tensor_copy
            copy_y = nc.vector.tensor_copy if swap_copy else nc.scalar.copy

            # software pipeline state: (psb_tile, yt_tile, quad_index)
            pending = None

            def mm2_flush(pend):
                psb_t, yt_t, q = pend
                pb = pspool_b.tile([128, quad, 128], F32)
                for j in range(quad):
                    nc.tensor.matmul(
                        pb[:, j, :],
                        lhsT=psb_t[:, j, :],
                        rhs=v_sb[:],
                        start=True,
                        stop=True,
                    )
                copy_y(yt_t[:, q * quad : (q + 1) * quad, :], pb[:])

            for g in range(n_tokens // group):
                xt = xpool.tile([128, group, 128], BF16)
                nc.sync.dma_start(xt[:], xs[:, g * group : (g + 1) * group, :])
                yt = ypool.tile([128, group, 128], BF16)
                for q in range(group // quad):
                    pa = pspool_a.tile([128, quad, 128], F32)
                    for j in range(quad):
                        nc.tensor.matmul(
                            pa[:, j, :],
                            lhsT=xt[:, q * quad + j, :],
                            rhs=u_sb[:],
                            start=True,
                            stop=True,
                        )
                    psb = ppool.tile([128, quad, 128], BF16)
                    copy_p(psb[:], pa[:])
                    if pipe:
                        if pending is not None:
                            mm2_flush(pending)
                        pending = (psb, yt, q)
                    else:
                        mm2_flush((psb, yt, q))
                # stores are emitted per group; the store for group g waits
                # on the yt copies via tile deps (the last quad's MM2 is
                # still pending under pipe, so its store region is written
                # one flush later -- handled below by flushing before store)
                if pipe and pending is not None and pending[1] is yt:
                    mm2_flush(pending)
                    pending = None
                nc.scalar.dma_start(
                    ys[:, g * group : (g + 1) * group, :], yt[:]
                )
    nc.compile()
    return nc


_NC_CACHE = {}


def _get_nc(**kw):
    key = tuple(sorted(kw.items()))
    if key not in _NC_CACHE:
        _NC_CACHE[key] = build_nc(**kw)
    return _NC_CACHE[key]


def _prep_inputs(x, U, V):
    """Host-side prep: cast to bf16 and permute x to [i1, T, i2]."""
    x = np.asarray(x, dtype=np.float32)
    U = np.asarray(U, dtype=np.float32)
    V = np.asarray(V, dtype=np.float32)
    t = x.shape[0]
    xs = np.ascontiguousarray(
        x.astype(NP_BF16).reshape(t, 128, 128).transpose(1, 0, 2)
    )
    return xs, U.astype(NP_BF16), V.astype(NP_BF16)


def run(x, U, V, group=32, quad=4, pipe=True, swap_copy=False,
        trace=False, **spmd_kwargs):
    """Shard over 8 cores, run, gather. Returns (y_full, BassKernelResults)."""
    xs, Ub, Vb = _prep_inputs(x, U, V)
    t_core = xs.shape[1] // N_CORES
    nc = _get_nc(n_tokens=t_core, group=group, quad=quad, pipe=pipe,
                 swap_copy=swap_copy)
    in_maps = [
        {"xs": np.ascontiguousarray(xs[:, i * t_core : (i + 1) * t_core, :]),
         "u": Ub, "v": Vb}
        for i in range(N_CORES)
    ]
    res = run_bass_kernel_spmd(
        nc, in_maps, list(range(N_CORES)), trace=trace, **spmd_kwargs
    )
    # ys[core] is [j1, t_core, j2] -> y[t, j1*128+j2]
    ys = np.stack([res.results[i]["ys"] for i in range(N_CORES)], axis=0)
    y = ys.transpose(0, 2, 1, 3).reshape(N_CORES * t_core, D)
    return y.astype(np.float32), res


def kernel(x, U, V):
    out, _ = run(x, U, V)
    return out
